# revision 1
# baseline (speedup 1.0000x reference)
"""BoundaryLoss Trainium2 kernel.

Data-parallel: one image of the batch of 8 per NeuronCore; the scalar
mean is reduced on the host.  Per-core:

  EDT: vertical column distance via fwd/bwd min-plus scans (DVE, fp16),
  cap at 16, square, PE-transpose, then a windowed parabola pass over
  |dx|<=3 (numpy-validated: rel err 6.7e-3 vs exact EDT; gate is 2e-2).
  The per-dx "+dx^2" tiles are built independently from the unmutated
  base (4x-mode tensor_scalar) so the min reduction is a shallow tree,
  not a serial chain.  Both masks are stacked in one tile.  sqrt via
  exp(0.5*ln(.)) keeps ACT in the one table set that holds Exp+Ln.
  w = 1 + 5*exp(-dist/3) computed per 128-row strip (strip 2 first,
  feeding the small final superblock), staged to DRAM as fp16.

  CE: pred cast to bf16 by the DMA into a [120 = 6 pixel-groups x 20
  channels, N] layout, in three uneven superblocks (20/20/8 chunks -
  the small last superblock shortens the pipeline drain).  exp(pred)
  in place on ACT; target DMA-replicated across the 20 channel
  partitions (stride-0 broadcast descriptors, bf16) so the one-hot is
  a 4x-mode tensor_scalar is_equal + 2x tensor_tensor mult against
  exp(pred); channel sums S = sum_c exp(p) and G = exp(p_target) both
  reduced on PE with a stationary block-diagonal ones matrix into one
  PSUM tile; ce = ln(S) - ln(G); final sum(w*ce) via fused
  scalar_tensor_tensor with accum.  The six big loads are interleaved
  on the serial DMA device via logical scheduler timestamps; a slice
  of elementwise work runs on the otherwise-idle Pool engine.
"""
from contextlib import ExitStack

import ml_dtypes
import numpy as np

import concourse.bass as bass
import concourse.mybir as mybir
from concourse import bacc, tile
from concourse import bass_utils
import concourse.bacc as _bacc_mod
from concourse.hw_specs import get_activation_tables as _gat


def _patched_tables(arch):
    # Force every activation function this kernel uses (Exp, Ln,
    # Identity) onto the one set that genuinely contains all of them, so
    # the chooser never inserts a mid-kernel table reload. Set ids keep
    # their positions; only membership changes (restrictively).
    tabs = _gat(arch)
    used = (mybir.ActivationFunctionType.Exp,
            mybir.ActivationFunctionType.Ln,
            mybir.ActivationFunctionType.Identity)
    both = [n for n, s in tabs.items() if all(f in s for f in used)]
    if both:
        keep = both[0]
        for n, s in tabs.items():
            if n != keep:
                for f in used:
                    s.discard(f)
    return tabs


_bacc_mod.get_activation_tables = _patched_tables

dt = mybir.dt
Alu = mybir.AluOpType
Act = mybir.ActivationFunctionType

N_CORES = 8
H = W = 384
HW = H * W              # 147456
C = 20
SBK = 3                 # superblocks per image (CE phase)
F = 512                 # chunk free size
G6 = 6                  # pixel groups stacked on partitions
CHKS = [20, 20, 8]      # matmul chunks per superblock (uneven: small tail)
JS = [c * F for c in CHKS]      # per-group free elems per superblock
QS = [G6 * c for c in CHKS]     # result-tile partitions per superblock
OFF = [0, G6 * JS[0], G6 * (JS[0] + JS[1])]  # flat pixel offsets
NECS = [4, 4, 2]        # exp/oh/gp chunks per superblock
XSH = G6 * (max(CHKS) - 1)      # ones_shift anchor column (114)
OSW = XSH + QS[0] + G6          # ones_shift width (240)
CAP = 16.0              # distance cap
BIGD = 300.0            # "infinite" 1d distance sentinel
RAD = 3                 # pass-2 window radius (approx; validated 6.7e-3)
PAD2 = RAD              # pass-2 x padding
THETA0 = 3.0
THETA = 5.0

_CACHED = {}

# logical scheduler timestamps (ms) for the six big CE loads, in order
# pred0, tbr0, pred1, tbr1, pred2, tbr2
DMA_STAMPS = [0.000, 0.011, 0.022, 0.030, 0.036, 0.040]


def _consts():
    ones_shift = np.zeros((120, OSW), np.float32)
    for g in range(G6):
        ones_shift[20 * g:20 * g + 20, XSH + g] = 1.0
    iota120 = np.tile(np.arange(C, dtype=np.float32), G6)[:, None]
    ident = np.eye(128, dtype=np.float16)
    return {
        "ones_shift": ones_shift.astype(ml_dtypes.bfloat16),
        "iota120": iota120,
        "ident": ident,
    }


def build_nc():
    nc = bacc.Bacc("TRN2", target_bir_lowering=False, debug=False,
                   num_devices=N_CORES)
    pred_d = nc.dram_tensor("pred", [C, H, W], dt.float32, kind="ExternalInput")
    tgt_d = nc.dram_tensor("target", [H, W], dt.int32, kind="ExternalInput")
    ones_d = nc.dram_tensor("ones_shift", [120, OSW], dt.bfloat16, kind="ExternalInput")
    iota_d = nc.dram_tensor("iota120", [120, 1], dt.float32, kind="ExternalInput")
    ident_d = nc.dram_tensor("ident", [128, 128], dt.float16, kind="ExternalInput")
    part_d = nc.dram_tensor("partial", [QS[0], 1], dt.float32, kind="ExternalOutput")

    with tile.TileContext(nc) as tc, ExitStack() as ctx:
        sb = ctx.enter_context(tc.tile_pool(name="sb", bufs=1))
        sb2 = ctx.enter_context(tc.tile_pool(name="sb2", bufs=2))
        ps = ctx.enter_context(
            tc.tile_pool(name="ps", bufs=2, space=bass.MemorySpace.PSUM))
        dr = ctx.enter_context(
            tc.tile_pool(name="dr", bufs=1, space=bass.MemorySpace.DRAM))

        # ---- tgt_nat heads the Pool DGE queue (it gates the whole EDT
        # front); non-casting consts ride the sync queue ----
        tgt_nat = sb.tile([128, SBK, W], dt.float16)
        nc.gpsimd.dma_start(
            tgt_nat[:], tgt_d.ap().rearrange("(sy y) x -> y sy x", sy=SBK))
        ident = sb.tile([128, 128], dt.float16)
        nc.sync.dma_start(ident[:], ident_d.ap())
        ones_shift = sb.tile([120, OSW], dt.bfloat16)
        nc.sync.dma_start(ones_shift[:], ones_d.ap())
        iota120 = sb.tile([120, 1], dt.float32)
        nc.sync.dma_start(iota120[:], iota_d.ap())
        pred_flat = pred_d.ap().rearrange("c y x -> c (y x)")
        pred_rs = [
            pred_flat[:, OFF[s]:OFF[s] + G6 * JS[s]].rearrange(
                "c (g j) -> g c j", g=G6, j=JS[s])
            for s in range(SBK)]
        pred_sbs = []
        for s in range(SBK):
            pred_sb = sb2.tile([120, JS[s]], dt.bfloat16, tag=f"pred{s}",
                               bufs=1, name=f"pred{s}")
            pred_sbs.append(pred_sb)

        delta_aps = {}
        for dx in range(1, RAD + 1):
            d_ap = sb.tile([128, 1], dt.float32, tag=f"delta{dx}",
                           name=f"delta{dx}")
            nc.gpsimd.memset(d_ap[:], float(dx * dx))
            delta_aps[dx] = d_ap
        eps_ap = sb.tile([128, 1], dt.float32)
        nc.gpsimd.memset(eps_ap[:], 1e-6)
        ones1 = sb.tile([128, H], dt.float16)
        nc.gpsimd.memset(ones1[:], 1.0)

        # --- CE input prefetch: t6 (tiny) unlocks the tbr replications;
        # tbr_s and pred_s transfers are interleaved on the serial DMA
        # device via logical wait timestamps (scheduler-order only) so
        # the DVE one-hot stream (tbr) and ACT exp stream (pred) both
        # start early and stay fed. ---
        # target in [(s g) (18 partitions), j] layout (bf16)
        tgt_flat = tgt_d.ap().rearrange("y x -> (y x)")
        t6 = sb.tile([G6 * SBK, JS[0]], dt.bfloat16)
        for s in range(SBK):
            nc.gpsimd.dma_start(
                t6[G6 * s:G6 * (s + 1), 0:JS[s]],
                tgt_flat[OFF[s]:OFF[s] + G6 * JS[s]].rearrange(
                    "(g j) -> g j", g=G6, j=JS[s]))
        tbrs = []
        for s in range(SBK):
            tbr = sb2.tile([120, JS[s]], dt.bfloat16, tag=f"tbr{s}",
                           bufs=1, name=f"tbr{s}")
            tbrs.append(tbr)
        dma_plan = list(zip(["pred", "tbr"] * SBK,
                            [0, 0, 1, 1, 2, 2],
                            DMA_STAMPS))
        for kind, s, ts in dma_plan:
            with tc.tile_wait_until(ts):
                if kind == "pred":
                    nc.gpsimd.dma_start(pred_sbs[s][:], pred_rs[s])
                else:
                    nc.sync.dma_start(
                        tbrs[s][:],
                        t6[G6 * s:G6 * (s + 1), 0:JS[s]].rearrange(
                            "g (o j) -> g o j", o=1).to_broadcast(
                                [G6, C, JS[s]]))

        # =========================== EDT ===========================
        with tc.high_priority():
            # PE-transpose to [x' (128), sx (3), y (384)]
            tgt_T = sb.tile([128, SBK, H], dt.float16)
            for sx in range(SBK):
                tp0 = ps.tile([128, SBK, 128], dt.float16, tag="tp")
                for sy in range(SBK):
                    nc.tensor.transpose(
                        tp0[:, sy, :], tgt_nat[:, sy, 128 * sx:128 * (sx + 1)],
                        ident[:])
                nc.vector.tensor_copy(
                    tgt_T[:, sx, :],
                    tp0[:].rearrange("p s x -> p (s x)"))

            # F fields for both masks stacked: [x', m (2), sx (3), y]
            Fst = sb.tile([128, 2, SBK, H], dt.float16)
            for m in (0, 1):
                nc.vector.tensor_scalar(Fst[:, m], tgt_T[:], float(m), -BIGD,
                                        op0=Alu.is_equal, op1=Alu.mult)
                nc.vector.tensor_scalar(Fst[:, m], Fst[:, m], BIGD, None,
                                        op0=Alu.add)
            # vertical fwd/bwd min-plus scans (Pool engine)
            for m in (0, 1):
                for s_ in range(SBK):
                    nc.vector.tensor_tensor_scan(
                        Fst[:, m, s_, :], ones1[:], Fst[:, m, s_, :], BIGD,
                        op0=Alu.add, op1=Alu.min)
                    nc.vector.tensor_tensor_scan(
                        Fst[:, m, s_, ::-1], ones1[:], Fst[:, m, s_, ::-1], BIGD,
                        op0=Alu.add, op1=Alu.min)
            # cap and square in place -> P = min(d1, CAP)^2
            nc.vector.tensor_scalar(Fst[:], Fst[:], CAP, None, op0=Alu.min)
            nc.vector.tensor_mul(Fst[:], Fst[:], Fst[:])

            # transpose to [y', m, sy, x_padded]
            X0, X1 = PAD2, PAD2 + W
            Dp = sb.tile([128, 2, SBK, W + 2 * PAD2], dt.float16)
            nc.gpsimd.memset(Dp[:, :, :, 0:X0], 2.0 * CAP * CAP)
            nc.gpsimd.memset(Dp[:, :, :, X1:], 2.0 * CAP * CAP)
            for m in (0, 1):
                for sy in range(SBK):
                    tp = ps.tile([128, SBK, 128], dt.float16, tag="tp")
                    for sx in range(SBK):
                        nc.tensor.transpose(
                            tp[:, sx, :], Fst[:, m, sx, 128 * sy:128 * (sy + 1)],
                            ident[:])
                    nc.vector.tensor_copy(
                        Dp[:, m, sy, X0:X1], tp[:].rearrange("p s x -> p (s x)"))

            # pass 2: exact windowed parabola min over |dx| <= RAD.
            # tmp_dx = Dp + dx^2 built independently on ACT; min tree on DVE.
            tmps = {}
            for dx in range(1, RAD + 1):
                t_dx = sb.tile([128, 2, SBK, W + 2 * PAD2], dt.float16,
                               tag=f"tmp{dx}", name=f"tmp{dx}")
                # all +dx^2 on DVE (4x tensor_scalar): any prioritized
                # ACT op here would head-of-line block the exp stream
                nc.vector.tensor_scalar(t_dx[:], Dp[:], float(dx * dx),
                                        None, op0=Alu.add)
                tmps[dx] = t_dx
            ms = {}
            for dx in range(1, RAD + 1):
                m_dx = sb.tile([128, 2, SBK, W], dt.float16, tag=f"m{dx}",
                               name=f"m{dx}")
                eng = nc.vector
                eng.tensor_tensor(
                    m_dx[:], tmps[dx][:, :, :, X0 - dx:X1 - dx],
                    tmps[dx][:, :, :, X0 + dx:X1 + dx], op=Alu.min)
                ms[dx] = m_dx
            # tree: (m1,m2), (.,m3), then center
            nc.vector.tensor_tensor(ms[1][:], ms[1][:], ms[2][:], op=Alu.min)
            nc.vector.tensor_tensor(ms[1][:], ms[1][:], ms[3][:], op=Alu.min)
            acc = sb.tile([128, 2, SBK, W], dt.float16)
            nc.vector.tensor_tensor(acc[:], ms[1][:], Dp[:, :, :, X0:X1],
                                    op=Alu.min)

        # sqrt via exp(0.5*ln(.)) computed in place on acc, then
        # w = 1 + THETA*exp(-dist/THETA0), per strip so the chains
        # --- superblock-0 exp chunks, emitted before the w chain so
        # they head the ACT queue (their input lands ~10us earlier than
        # acc; emission order fixes the in-order ACT queue) ---
        for k in range(NECS[0]):
            EC0 = JS[0] // NECS[0]
            ck = slice(k * EC0, (k + 1) * EC0)
            nc.scalar.activation(pred_sbs[0][:, ck], pred_sbs[0][:, ck],
                                 Act.Exp)

        # --- w chain: sqrt via exp(0.5*ln(.)) in place on acc, then
        # w = 1 + THETA*exp(-dist/THETA0), per strip (pipelines through
        # ACT/DVE). Strip 2 first: the small final superblock (the
        # kernel's tail) reads only strip 2 of w.
        w_dr = dr.tile([HW], dt.float16)
        w_img = w_dr[:].rearrange("(sy y x) -> sy y x", sy=SBK, y=128,
                                  x=W)
        dist = sb.tile([128, SBK, W], dt.float16)
        for sy in (2, 1, 0):
            nc.scalar.activation(acc[:, :, sy, :], acc[:, :, sy, :],
                                 Act.Ln, bias=eps_ap[:])
            nc.scalar.activation(acc[:, :, sy, :], acc[:, :, sy, :],
                                 Act.Exp, scale=0.5)
            nc.vector.tensor_add(dist[:, sy, :], acc[:, 0, sy, :],
                                 acc[:, 1, sy, :])
            nc.scalar.activation(dist[:, sy, :], dist[:, sy, :], Act.Exp,
                                 scale=-1.0 / THETA0)
            nc.vector.tensor_scalar(dist[:, sy, :], dist[:, sy, :],
                                    THETA, 1.0, op0=Alu.mult, op1=Alu.add)
            nc.sync.dma_start(w_img[sy], dist[:, sy, :])

        # =========================== CE ===========================
        # per sb: pixel = OFF[s] + (g*CHKS[s] + i)*F + f ; result
        # partition q = 6*i + g
        acc_prev = None
        for s in range(SBK):
            pred_sb = pred_sbs[s]
            tbr = tbrs[s]
            Qs, CHK = QS[s], CHKS[s]

            # per chunk: exp in place over pred (ACT; pred tile doubles
            # as expp), one-hot in place over tbr (4x mode,
            # independent), then oh*exp(pred) in place over tbr (2x).
            # G = sum_c oh*exp(p) = exp(p_t), so ce = ln(S/G).
            # Chunking lets the PE matmuls unblock progressively.
            NEC = NECS[s]
            EC = JS[s] // NEC
            for k in range(NEC):
                ck = slice(k * EC, (k + 1) * EC)
                if s > 0:
                    nc.scalar.activation(pred_sb[:, ck], pred_sb[:, ck],
                                         Act.Exp)
                nc.vector.tensor_scalar(tbr[:, ck], tbr[:, ck],
                                        iota120[:], None, op0=Alu.is_equal)
                # one mid-stream gather-product chunk per early
                # superblock runs on the idle Pool engine to offload
                # DVE — none for the last superblock (the kernel's
                # tail), and not the last chunk (it gates the final
                # G matmuls of its superblock)
                eng = nc.gpsimd if (k == 1 and s < SBK - 1) \
                    else nc.vector
                eng.tensor_tensor(tbr[:, ck], tbr[:, ck],
                                  pred_sb[:, ck], op=Alu.mult)

            # channel sums on PE into one PSUM tile: S -> [:,0,:],
            # G = exp(p_t) -> [:,1,:]
            sg_ps = ps.tile([Qs, 2, F], dt.float32, tag=f"sg{s}", bufs=1)
            for i in range(CHK):
                osl = ones_shift[:, XSH - 6 * i:XSH - 6 * i + Qs]
                nc.tensor.matmul(sg_ps[:, 0, :], osl,
                                 pred_sb[:, i * F:(i + 1) * F],
                                 start=(i == 0), stop=(i == CHK - 1))
            for i in range(CHK):
                osl = ones_shift[:, XSH - 6 * i:XSH - 6 * i + Qs]
                nc.tensor.matmul(sg_ps[:, 1, :], osl,
                                 tbr[:, i * F:(i + 1) * F],
                                 start=(i == 0), stop=(i == CHK - 1))

            lsg = sb2.tile([Qs, 2, F], dt.float32, tag=f"lsg{s}", bufs=1)
            nc.scalar.activation(lsg[:], sg_ps[:], Act.Ln)
            ce_t = sb2.tile([Qs, F], dt.float32, tag=f"cet{s}", bufs=1)
            sub_eng = nc.gpsimd if s < SBK - 1 else nc.vector
            sub_eng.tensor_tensor(ce_t[:], lsg[:, 0, :], lsg[:, 1, :],
                                  op=Alu.subtract)

            w_sb = sb2.tile([Qs, F], dt.float16, tag=f"wsb{s}", bufs=1)
            nc.sync.dma_start(
                w_sb[:],
                w_dr[OFF[s]:OFF[s] + G6 * JS[s]].rearrange(
                    "(g i f) -> i g f", g=G6, i=CHK, f=F))
            junk = sb2.tile([Qs, F], dt.float32, tag=f"junk{s}", bufs=1)
            acc_t = sb.tile([Qs, 1], dt.float32, tag=f"acc{s}",
                            name=f"acc{s}")
            nc.vector.scalar_tensor_tensor(
                junk[:], ce_t[:], 1.0, w_sb[:],
                op0=Alu.mult, op1=Alu.mult, accum_out=acc_t[:])
            if acc_prev is None:
                acc_prev = acc_t
            elif Qs == QS[0]:
                nc.vector.tensor_add(acc_t[:], acc_t[:], acc_prev[:])
                acc_prev = acc_t
            else:
                nc.vector.tensor_add(acc_prev[0:Qs, :],
                                     acc_prev[0:Qs, :], acc_t[:])

        nc.sync.dma_start(part_d.ap(), acc_prev[:])

    nc.compile()
    return nc


def kernel(pred, target):
    key = "nc"
    if key not in _CACHED:
        _CACHED[key] = build_nc()
    nc = _CACHED[key]
    consts = _consts()
    in_maps = []
    for b in range(N_CORES):
        in_maps.append({
            "pred": np.ascontiguousarray(pred[b], dtype=np.float32),
            "target": np.ascontiguousarray(target[b], dtype=np.int32),
            "ones_shift": consts["ones_shift"],
            "iota120": consts["iota120"],
            "ident": consts["ident"],
        })
    res = bass_utils.run_bass_kernel_spmd(
        nc, in_maps, core_ids=list(range(N_CORES)))
    total = 0.0
    for b in range(N_CORES):
        total += float(res.results[b]["partial"].astype(np.float64).sum())
    return np.float32(total / (N_CORES * HW))



# revision 5
# speedup vs baseline: 1.0381x; 1.0381x over previous
"""BoundaryLoss Trainium2 kernel (v2).

Data-parallel: one image of the batch of 8 per NeuronCore; the scalar
mean is reduced on the host.  Per-core design notes (cost model: DMA
charges destination bytes on ONE serial device; engine ops charge
per-partition free-size cycles only):

  CE: pred DMA-cast f32->fp8e4m3 (halves the dominant pred DMA traffic;
  numpy-validated rel err 6.9e-3 vs gate 2e-2).  exp on ACT reads fp8,
  writes bf16 expp.  target replicated across the 20 channel partitions
  as bf16 (stride-0 broadcast, split in halves for earlier consumption);
  one-hot via 4x-mode tensor_scalar is_equal in place, product with expp
  via 2x tensor_tensor (some chunks on the idle Pool engine).  Channel
  sums S (from expp) and G = exp(p_t) (from oh*expp) via PE matmuls with
  a sliding block-diagonal ones stationary into one PSUM tile.
  ce = ln(S/G): DVE divide then ONE ACT Ln whose accum_out yields
  sum(ce) per partition for free; final sum(5*u*ce) via
  scalar_tensor_tensor accum, so w = 1+5u is never materialized.

  EDT: tgt in [y,(sy,x)] fp16, PE-transpose, F-field built directly from
  transpose PSUM via ONE fused not_equal*CAP tensor_scalar per (m,sx)
  (cap 16 baked into the field so the scan self-caps).  Vertical
  fwd/bwd min-plus scans as TWO whole-tile scans (6 strips each) using
  boundary-reset columns in data0 (fwd on DVE, bwd on Pool).  Square is
  fused into the PSUM->SBUF copy after the second transpose (tp*tp).
  Windowed parabola |dx|<=3 (3 pair-mins + 3 adds + 3 tree-mins).
  sqrt via exp(0.5*ln(D) - ln3) = sqrt(D)/3 on ACT (one table set holds
  Exp+Ln), u = exp(-(d0+d1)) stored fp8 through a DRAM round trip to the
  CE pixel-group layout.

  DMA order shaped with logical scheduler timestamps: tgt, t6, then
  pred_s (fp8, Pool SWDGE) interleaved with tbr halves (bf16, SP HWDGE).
"""
from contextlib import ExitStack

import ml_dtypes
import numpy as np

import concourse.bass as bass
import concourse.mybir as mybir
from concourse import bacc, tile
from concourse import bass_utils
import concourse.bacc as _bacc_mod
from concourse.hw_specs import get_activation_tables as _gat


def _patched_tables(arch):
    # Force every activation function this kernel uses (Exp, Ln, Identity)
    # onto the one set that genuinely contains all of them, so the chooser
    # never inserts a mid-kernel table reload.
    tabs = _gat(arch)
    used = (mybir.ActivationFunctionType.Exp,
            mybir.ActivationFunctionType.Ln,
            mybir.ActivationFunctionType.Identity)
    both = [n for n, s in tabs.items() if all(f in s for f in used)]
    if both:
        keep = both[0]
        for n, s in tabs.items():
            if n != keep:
                for f in used:
                    s.discard(f)
    return tabs


_bacc_mod.get_activation_tables = _patched_tables

dt = mybir.dt
Alu = mybir.AluOpType
Act = mybir.ActivationFunctionType

N_CORES = 8
H = W = 384
HW = H * W              # 147456
C = 20
SBK = 3                 # superblocks (CE phase)
F = 512                 # matmul moving chunk
G6 = 6                  # pixel groups stacked on partitions
CHK = 16                # matmul chunks per superblock
JS = CHK * F            # 8192 per-group free elems per superblock
JS2 = JS // 2
Qs = G6 * CHK           # 96 result partitions
OFF = [0, G6 * JS, 2 * G6 * JS]
NEC = 4                 # exp/oh chunks per superblock
EC = JS // NEC          # 2048
XSH = G6 * (CHK - 1)    # ones_shift anchor column (90)
OSW = XSH + Qs + G6     # ones_shift width (192)
CAP = 16.0              # distance cap (baked into the F field)
RESET = 99.0            # scan boundary reset (> CAP)
PADV = 2.0 * CAP * CAP  # x-pad sentinel for pass 2
RAD = 3                 # pass-2 window radius
LN3 = float(np.log(3.0))

_CACHED = {}

# logical scheduler timestamps (ms) shaping the serial DMA device order
ST_TGT = 0.0
ST_T6 = 0.0008
ST_PRED = [0.0017, 0.0101, 0.0185]
ST_TBR = [(0.0045, 0.0073), (0.0129, 0.0157), (0.0213, 0.0241)]
ST_UW = 0.0196
ST_UR = [0.0199, 0.0200, 0.0201]

# (s, k) chunks whose oh*expp product runs on the Pool engine
POOL_MULT = {(0, 1), (0, 3), (1, 1), (2, 1)}


def _consts():
    ones_shift = np.zeros((120, OSW), np.float32)
    for g in range(G6):
        ones_shift[20 * g:20 * g + 20, XSH + g] = 1.0
    iota120 = np.tile(np.arange(C, dtype=np.float32), G6)[:, None]
    ident = np.eye(128, dtype=np.float16)
    return {
        "ones_shift": ones_shift.astype(ml_dtypes.bfloat16),
        "iota120": iota120,
        "ident": ident,
    }


def build_nc():
    nc = bacc.Bacc("TRN2", target_bir_lowering=False, debug=False,
                   num_devices=N_CORES)
    pred_d = nc.dram_tensor("pred", [C, H, W], dt.float32, kind="ExternalInput")
    tgt_d = nc.dram_tensor("target", [H, W], dt.int32, kind="ExternalInput")
    ones_d = nc.dram_tensor("ones_shift", [120, OSW], dt.bfloat16, kind="ExternalInput")
    iota_d = nc.dram_tensor("iota120", [120, 1], dt.float32, kind="ExternalInput")
    ident_d = nc.dram_tensor("ident", [128, 128], dt.float16, kind="ExternalInput")
    part_d = nc.dram_tensor("partial", [Qs, 1], dt.float32, kind="ExternalOutput")

    with tile.TileContext(nc) as tc, ExitStack() as ctx:
        sb = ctx.enter_context(tc.tile_pool(name="sb", bufs=1))
        sb2 = ctx.enter_context(tc.tile_pool(name="sb2", bufs=2))
        ps = ctx.enter_context(
            tc.tile_pool(name="ps", bufs=2, space=bass.MemorySpace.PSUM))
        dr = ctx.enter_context(
            tc.tile_pool(name="dr", bufs=1, space=bass.MemorySpace.DRAM))

        # ---- tgt_nat heads the Pool DGE queue (gates the EDT front) ----
        tgt_nat = sb.tile([128, SBK, W], dt.float16)
        with tc.tile_wait_until(ST_TGT):
            nc.gpsimd.dma_start(
                tgt_nat[:], tgt_d.ap().rearrange("(sy y) x -> y sy x", sy=SBK))
        ident = sb.tile([128, 128], dt.float16)
        nc.sync.dma_start(ident[:], ident_d.ap())
        ones_shift = sb.tile([120, OSW], dt.bfloat16)
        nc.sync.dma_start(ones_shift[:], ones_d.ap())
        iota120 = sb.tile([120, 1], dt.float32)
        nc.sync.dma_start(iota120[:], iota_d.ap())

        eps_ap = sb.tile([128, 1], dt.float32)
        nc.gpsimd.memset(eps_ap[:], 1e-6)
        ln3_ap = sb.tile([128, 1], dt.float32)
        nc.gpsimd.memset(ln3_ap[:], -LN3)
        # scan data0 tiles: ones with RESET at each strip's first (fwd) /
        # last (bwd) column; 6 strips of 384 along the free dim
        ones_f = sb.tile([128, 6, W], dt.float16)
        nc.gpsimd.memset(ones_f[:], 1.0)
        nc.gpsimd.memset(ones_f[:, :, 0:1], RESET)
        ones_b = sb.tile([128, 6, W], dt.float16)
        nc.gpsimd.memset(ones_b[:], 1.0)
        nc.gpsimd.memset(ones_b[:, :, W - 1:W], RESET)

        # ---- CE input loads ----
        tgt_flat = tgt_d.ap().rearrange("y x -> (y x)")
        t6 = sb.tile([SBK * G6, JS], dt.bfloat16)
        with tc.tile_wait_until(ST_T6):
            nc.gpsimd.dma_start(
                t6[:], tgt_flat.rearrange("(s g j) -> (s g) j", s=SBK, g=G6, j=JS))

        pred_flat = pred_d.ap().rearrange("c y x -> c (y x)")
        pred_sbs = []
        for s in range(SBK):
            pred_sb = sb2.tile([120, JS], dt.float8e4, tag=f"pred{s}",
                               bufs=1, name=f"pred{s}")
            pred_sbs.append(pred_sb)
            with tc.tile_wait_until(ST_PRED[s]):
                nc.gpsimd.dma_start(
                    pred_sb[:],
                    pred_flat[:, OFF[s]:OFF[s] + G6 * JS].rearrange(
                        "c (g j) -> g c j", g=G6, j=JS))
        tbrs = []
        for s in range(SBK):
            tbr = sb2.tile([120, JS], dt.bfloat16, tag=f"tbr{s}",
                           bufs=1, name=f"tbr{s}")
            tbrs.append(tbr)
            for h in range(2):
                hs = slice(h * JS2, (h + 1) * JS2)
                with tc.tile_wait_until(ST_TBR[s][h]):
                    nc.sync.dma_start(
                        tbr[:, hs],
                        t6[G6 * s:G6 * (s + 1), hs].rearrange(
                            "g (o j) -> g o j", o=1).to_broadcast(
                                [G6, C, JS2]))

        # =========================== EDT ===========================
        with tc.high_priority():
            # stage-1 transposes + fused F-build straight from PSUM:
            # Fst[x', m, sx, y] = CAP * (t != m)
            Fst = sb.tile([128, 2, SBK, H], dt.float16)
            for sx in range(SBK):
                tp0 = ps.tile([128, SBK, 128], dt.float16, tag="tp")
                for sy in range(SBK):
                    nc.tensor.transpose(
                        tp0[:, sy, :], tgt_nat[:, sy, 128 * sx:128 * (sx + 1)],
                        ident[:])
                tpf = tp0[:].rearrange("p s y -> p (s y)")
                for m in (0, 1):
                    nc.vector.tensor_scalar(Fst[:, m, sx, :], tpf, float(m),
                                            CAP, op0=Alu.not_equal, op1=Alu.mult)

            # vertical fwd/bwd min-plus scans over all 6 strips at once;
            # data0 boundary columns reset the running state per strip
            F2 = Fst[:].rearrange("p m s y -> p (m s y)")
            of = ones_f[:].rearrange("p s y -> p (s y)")
            ob = ones_b[:].rearrange("p s y -> p (s y)")
            nc.vector.tensor_tensor_scan(F2, of, F2, RESET,
                                         op0=Alu.add, op1=Alu.min)
            nc.gpsimd.tensor_tensor_scan(F2[:, ::-1], ob[:, ::-1], F2[:, ::-1],
                                         RESET, op0=Alu.add, op1=Alu.min)

            # stage-2 transposes; square fused into the PSUM->SBUF copy
            X0, X1 = RAD, RAD + W
            Dp = sb.tile([128, 2, SBK, W + 2 * RAD], dt.float16)
            nc.gpsimd.memset(Dp[:, :, :, 0:X0], PADV)
            nc.gpsimd.memset(Dp[:, :, :, X1:], PADV)
            for m in (0, 1):
                for sy in range(SBK):
                    tp = ps.tile([128, SBK, 128], dt.float16, tag="tp")
                    for sx in range(SBK):
                        nc.tensor.transpose(
                            tp[:, sx, :], Fst[:, m, sx, 128 * sy:128 * (sy + 1)],
                            ident[:])
                    tpf = tp[:].rearrange("p s x -> p (s x)")
                    nc.vector.tensor_tensor(Dp[:, m, sy, X0:X1], tpf, tpf,
                                            op=Alu.mult)

            # pass 2: windowed parabola min over |dx| <= 3 (exact)
            ms = {}
            for dx in range(1, RAD + 1):
                m_dx = sb.tile([128, 2, SBK, W], dt.float16, tag=f"m{dx}",
                               name=f"m{dx}")
                nc.vector.tensor_tensor(
                    m_dx[:], Dp[:, :, :, X0 - dx:X1 - dx],
                    Dp[:, :, :, X0 + dx:X1 + dx], op=Alu.min)
                ms[dx] = m_dx
            for dx in range(1, RAD + 1):
                nc.vector.tensor_scalar(ms[dx][:], ms[dx][:], float(dx * dx),
                                        None, op0=Alu.add)
            nc.vector.tensor_tensor(ms[1][:], ms[1][:], ms[2][:], op=Alu.min)
            nc.vector.tensor_tensor(ms[1][:], ms[1][:], ms[3][:], op=Alu.min)
            nc.vector.tensor_tensor(ms[1][:], ms[1][:], Dp[:, :, :, X0:X1],
                                    op=Alu.min)

        # sqrt(D)/3 via exp(0.5*ln(D) - ln3); u = exp(-(d0+d1)) in fp8
        acc = ms[1]
        nc.scalar.activation(acc[:], acc[:], Act.Ln, bias=eps_ap[:])
        nc.scalar.activation(acc[:], acc[:], Act.Exp, scale=0.5, bias=ln3_ap[:])
        dist = sb.tile([128, SBK, W], dt.float16)
        nc.vector.tensor_add(dist[:], acc[:, 0], acc[:, 1])
        u_t = sb.tile([128, SBK, W], dt.float8e4)
        nc.scalar.activation(u_t[:], dist[:], Act.Exp, scale=-1.0)
        u_dr = dr.tile([HW], dt.float8e4)
        with tc.tile_wait_until(ST_UW):
            nc.sync.dma_start(
                u_dr[:].rearrange("(sy y x) -> y sy x", sy=SBK, y=128, x=W),
                u_t[:])

        # =========================== CE ===========================
        acc_tot = None
        for s in range(SBK):
            pred_sb = pred_sbs[s]
            tbr = tbrs[s]
            expp = sb2.tile([120, JS], dt.bfloat16, tag="expp", bufs=2,
                            name=f"expp{s}")
            sg = ps.tile([Qs, 2, F], dt.float32, tag=f"sg{s}", bufs=1)
            for k in range(NEC):
                ck = slice(k * EC, (k + 1) * EC)
                nc.scalar.activation(expp[:, ck], pred_sb[:, ck], Act.Exp)
                nc.vector.tensor_scalar(tbr[:, ck], tbr[:, ck], iota120[:],
                                        None, op0=Alu.is_equal)
                eng = nc.gpsimd if (s, k) in POOL_MULT else nc.vector
                eng.tensor_tensor(tbr[:, ck], tbr[:, ck], expp[:, ck],
                                  op=Alu.mult)
                # matmuls for the F-chunks covered by this exp/oh chunk
                for i in range(k * (CHK // NEC), (k + 1) * (CHK // NEC)):
                    osl = ones_shift[:, XSH - G6 * i:XSH - G6 * i + Qs]
                    nc.tensor.matmul(sg[:, 0, :], osl,
                                     expp[:, i * F:(i + 1) * F],
                                     start=(i == 0), stop=(i == CHK - 1))
                    nc.tensor.matmul(sg[:, 1, :], osl,
                                     tbr[:, i * F:(i + 1) * F],
                                     start=(i == 0), stop=(i == CHK - 1))

            r_t = sb2.tile([Qs, F], dt.float32, tag="rt", bufs=2,
                           name=f"rt{s}")
            nc.vector.tensor_tensor(r_t[:], sg[:, 0, :], sg[:, 1, :],
                                    op=Alu.divide)
            ce_t = sb2.tile([Qs, F], dt.float32, tag="cet", bufs=2,
                            name=f"cet{s}")
            lnacc = sb.tile([Qs, 1], dt.float32, tag=f"lnacc{s}",
                            name=f"lnacc{s}")
            nc.scalar.activation(ce_t[:], r_t[:], Act.Ln, accum_out=lnacc[:])

            u_sb = sb2.tile([Qs, F], dt.float8e4, tag="usb", bufs=2,
                            name=f"usb{s}")
            with tc.tile_wait_until(ST_UR[s]):
                nc.sync.dma_start(
                    u_sb[:],
                    u_dr[OFF[s]:OFF[s] + G6 * JS].rearrange(
                        "(g i f) -> i g f", g=G6, i=CHK, f=F))
            junk = sb2.tile([Qs, F], dt.float32, tag="junk", bufs=2,
                            name=f"junk{s}")
            acc_s = sb.tile([Qs, 1], dt.float32, tag=f"acc{s}",
                            name=f"acc{s}")
            nc.vector.scalar_tensor_tensor(
                junk[:], ce_t[:], 5.0, u_sb[:],
                op0=Alu.mult, op1=Alu.mult, accum_out=acc_s[:])
            nc.vector.tensor_add(acc_s[:], acc_s[:], lnacc[:])
            if acc_tot is None:
                acc_tot = acc_s
            else:
                nc.vector.tensor_add(acc_tot[:], acc_tot[:], acc_s[:])

        nc.sync.dma_start(part_d.ap(), acc_tot[:])

    nc.compile()
    return nc


def kernel(pred, target):
    key = "nc"
    if key not in _CACHED:
        _CACHED[key] = build_nc()
    nc = _CACHED[key]
    consts = _consts()
    in_maps = []
    for b in range(N_CORES):
        in_maps.append({
            "pred": np.ascontiguousarray(pred[b], dtype=np.float32),
            "target": np.ascontiguousarray(target[b], dtype=np.int32),
            "ones_shift": consts["ones_shift"],
            "iota120": consts["iota120"],
            "ident": consts["ident"],
        })
    res = bass_utils.run_bass_kernel_spmd(
        nc, in_maps, core_ids=list(range(N_CORES)))
    total = 0.0
    for b in range(N_CORES):
        total += float(res.results[b]["partial"].astype(np.float64).sum())
    return np.float32(total / (N_CORES * HW))


# revision 9
# speedup vs baseline: 1.0977x; 1.0574x over previous
"""BoundaryLoss Trainium2 kernel (v2).

Data-parallel: one image of the batch of 8 per NeuronCore; the scalar
mean is reduced on the host.  Per-core design notes (cost model: DMA
charges destination bytes on ONE serial device; engine ops charge
per-partition free-size cycles only):

  CE: pred DMA-cast f32->fp8e4m3 (halves the dominant pred DMA traffic;
  numpy-validated rel err 6.9e-3 vs gate 2e-2).  exp on ACT reads fp8,
  writes bf16 expp.  target replicated across the 20 channel partitions
  as bf16 (stride-0 broadcast, split in halves for earlier consumption);
  one-hot via 4x-mode tensor_scalar is_equal in place, product with expp
  via 2x tensor_tensor (some chunks on the idle Pool engine).  Channel
  sums S (from expp) and G = exp(p_t) (from oh*expp) via PE matmuls with
  a sliding block-diagonal ones stationary into one PSUM tile.
  ce = ln(S/G): DVE divide then ONE ACT Ln whose accum_out yields
  sum(ce) per partition for free; final sum(5*u*ce) via
  scalar_tensor_tensor accum, so w = 1+5u is never materialized.

  EDT: tgt in [y,(sy,x)] fp16, PE-transpose, F-field built directly from
  transpose PSUM via ONE fused not_equal*CAP tensor_scalar per (m,sx)
  (cap 16 baked into the field so the scan self-caps).  Vertical
  fwd/bwd min-plus scans as TWO whole-tile scans (6 strips each) using
  boundary-reset columns in data0 (fwd on DVE, bwd on Pool).  Square is
  fused into the PSUM->SBUF copy after the second transpose (tp*tp).
  Windowed parabola |dx|<=3 (3 pair-mins + 3 adds + 3 tree-mins).
  sqrt via exp(0.5*ln(D) - ln3) = sqrt(D)/3 on ACT (one table set holds
  Exp+Ln), u = exp(-(d0+d1)) stored fp8 through a DRAM round trip to the
  CE pixel-group layout.

  DMA order shaped with logical scheduler timestamps: tgt, t6, then
  pred_s (fp8, Pool SWDGE) interleaved with tbr halves (bf16, SP HWDGE).
"""
from contextlib import ExitStack

import ml_dtypes
import numpy as np

import concourse.bass as bass
import concourse.mybir as mybir
from concourse import bacc, tile
from concourse import bass_utils
import concourse.bacc as _bacc_mod
from concourse.hw_specs import get_activation_tables as _gat


def _patched_tables(arch):
    # Force every activation function this kernel uses (Exp, Ln, Identity)
    # onto the one set that genuinely contains all of them, so the chooser
    # never inserts a mid-kernel table reload.
    tabs = _gat(arch)
    used = (mybir.ActivationFunctionType.Exp,
            mybir.ActivationFunctionType.Ln,
            mybir.ActivationFunctionType.Identity)
    both = [n for n, s in tabs.items() if all(f in s for f in used)]
    if both:
        keep = both[0]
        for n, s in tabs.items():
            if n != keep:
                for f in used:
                    s.discard(f)
    return tabs


_bacc_mod.get_activation_tables = _patched_tables

dt = mybir.dt
Alu = mybir.AluOpType
Act = mybir.ActivationFunctionType

N_CORES = 8
H = W = 384
HW = H * W              # 147456
C = 20
SBK = 3                 # superblocks (CE phase)
F = 512                 # matmul moving chunk
G6 = 6                  # pixel groups stacked on partitions
CHK = 16                # matmul chunks per superblock
JS = CHK * F            # 8192 per-group free elems per superblock
JS2 = JS // 2
Qs = G6 * CHK           # 96 result partitions
OFF = [0, G6 * JS, 2 * G6 * JS]
NEC = 4                 # exp/oh chunks per superblock
EC = JS // NEC          # 2048
XSH = G6 * (CHK - 1)    # ones_shift anchor column (90)
OSW = XSH + Qs + G6     # ones_shift width (192)
CAP = 16.0              # distance cap (baked into the F field)
RESET = 99.0            # scan boundary reset (> CAP)
PADV = 2.0 * CAP * CAP  # x-pad sentinel for pass 2
RAD = 3                 # pass-2 window radius
LN3 = float(np.log(3.0))

_CACHED = {}

# logical scheduler timestamps (ms) shaping the serial DMA device order
ST_TGT = 0.0
ST_T6 = 0.0008
ST_PRED = [0.0017, 0.0101, 0.0185]
ST_TBR = [(0.0045, 0.0073), (0.0129, 0.0157), (0.0213, 0.0241)]
ST_UW = 0.0196
ST_UR = [0.0199, 0.0200, 0.0201]

# (s, k) chunks whose oh*expp product runs on the Pool engine
POOL_MULT = {(0, 1), (0, 3), (1, 1), (2, 1)}


def _consts():
    ones_shift = np.zeros((120, OSW), np.float32)
    for g in range(G6):
        ones_shift[20 * g:20 * g + 20, XSH + g] = 1.0
    iota120 = np.tile(np.arange(C, dtype=np.float32), G6)[:, None]
    ident = np.eye(128, dtype=np.float16)
    return {
        "ones_shift": ones_shift.astype(ml_dtypes.bfloat16),
        "iota120": iota120,
        "ident": ident,
    }


def build_nc():
    nc = bacc.Bacc("TRN2", target_bir_lowering=False, debug=False,
                   num_devices=N_CORES)
    pred_d = nc.dram_tensor("pred", [C, H, W], dt.float32, kind="ExternalInput")
    tgt_d = nc.dram_tensor("target", [H, W], dt.int32, kind="ExternalInput")
    ones_d = nc.dram_tensor("ones_shift", [120, OSW], dt.bfloat16, kind="ExternalInput")
    iota_d = nc.dram_tensor("iota120", [120, 1], dt.float32, kind="ExternalInput")
    ident_d = nc.dram_tensor("ident", [128, 128], dt.float16, kind="ExternalInput")
    part_d = nc.dram_tensor("partial", [Qs, 1], dt.float32, kind="ExternalOutput")

    with tile.TileContext(nc) as tc, ExitStack() as ctx:
        sb = ctx.enter_context(tc.tile_pool(name="sb", bufs=1))
        sb2 = ctx.enter_context(tc.tile_pool(name="sb2", bufs=2))
        ps = ctx.enter_context(
            tc.tile_pool(name="ps", bufs=2, space=bass.MemorySpace.PSUM))
        dr = ctx.enter_context(
            tc.tile_pool(name="dr", bufs=1, space=bass.MemorySpace.DRAM))

        # ---- tgt_nat heads the Pool DGE queue (gates the EDT front) ----
        tgt_nat = sb.tile([128, SBK, W], dt.float16)
        with tc.tile_wait_until(ST_TGT):
            nc.gpsimd.dma_start(
                tgt_nat[:], tgt_d.ap().rearrange("(sy y) x -> y sy x", sy=SBK))
        ident = sb.tile([128, 128], dt.float16)
        nc.sync.dma_start(ident[:], ident_d.ap())
        ones_shift = sb.tile([120, OSW], dt.bfloat16)
        nc.sync.dma_start(ones_shift[:], ones_d.ap())
        iota120 = sb.tile([120, 1], dt.float32)
        nc.sync.dma_start(iota120[:], iota_d.ap())

        eps_ap = sb.tile([128, 1], dt.float32)
        nc.gpsimd.memset(eps_ap[:], 1e-6)
        ln3_ap = sb.tile([128, 1], dt.float32)
        nc.gpsimd.memset(ln3_ap[:], -LN3)
        # scan data0 tiles: ones with RESET at each strip's first (fwd) /
        # last (bwd) column; 6 strips of 384 along the free dim
        ones_f = sb.tile([128, 6, W], dt.float16)
        nc.gpsimd.memset(ones_f[:], 1.0)
        nc.gpsimd.memset(ones_f[:, :, 0:1], RESET)
        ones_b = sb.tile([128, 6, W], dt.float16)
        nc.gpsimd.memset(ones_b[:], 1.0)
        nc.gpsimd.memset(ones_b[:, :, W - 1:W], RESET)

        # ---- CE input loads ----
        tgt_flat = tgt_d.ap().rearrange("y x -> (y x)")
        t6 = sb.tile([SBK * G6, JS], dt.bfloat16)
        with tc.tile_wait_until(ST_T6):
            nc.gpsimd.dma_start(
                t6[:], tgt_flat.rearrange("(s g j) -> (s g) j", s=SBK, g=G6, j=JS))

        pred_flat = pred_d.ap().rearrange("c y x -> c (y x)")
        pred_sbs = []
        for s in range(SBK):
            pred_sb = sb2.tile([120, JS], dt.float8e4, tag=f"pred{s}",
                               bufs=1, name=f"pred{s}")
            pred_sbs.append(pred_sb)
            with tc.tile_wait_until(ST_PRED[s]):
                nc.gpsimd.dma_start(
                    pred_sb[:],
                    pred_flat[:, OFF[s]:OFF[s] + G6 * JS].rearrange(
                        "c (g j) -> g c j", g=G6, j=JS))
        tbrs = []
        for s in range(SBK):
            tbr = sb2.tile([120, JS], dt.bfloat16, tag=f"tbr{s}",
                           bufs=1, name=f"tbr{s}")
            tbrs.append(tbr)
            for h in range(2):
                hs = slice(h * JS2, (h + 1) * JS2)
                with tc.tile_wait_until(ST_TBR[s][h]):
                    nc.sync.dma_start(
                        tbr[:, hs],
                        t6[G6 * s:G6 * (s + 1), hs].rearrange(
                            "g (o j) -> g o j", o=1).to_broadcast(
                                [G6, C, JS2]))

        # =========================== EDT ===========================
        with tc.high_priority():
            # stage-1 transposes + fused F-build straight from PSUM:
            # Fst[x', m, sx, y] = CAP * (t != m)
            Fst = sb.tile([128, 2, SBK, H], dt.float16)
            for sx in range(SBK):
                tp0 = ps.tile([128, SBK, 128], dt.float16, tag="tp")
                for sy in range(SBK):
                    nc.tensor.transpose(
                        tp0[:, sy, :], tgt_nat[:, sy, 128 * sx:128 * (sx + 1)],
                        ident[:])
                tpf = tp0[:].rearrange("p s y -> p (s y)")
                for m in (0, 1):
                    nc.vector.tensor_scalar(Fst[:, m, sx, :], tpf, float(m),
                                            CAP, op0=Alu.not_equal, op1=Alu.mult)

            # vertical fwd/bwd min-plus scans over all 6 strips at once;
            # data0 boundary columns reset the running state per strip
            F2 = Fst[:].rearrange("p m s y -> p (m s y)")
            of = ones_f[:].rearrange("p s y -> p (s y)")
            ob = ones_b[:].rearrange("p s y -> p (s y)")
            nc.vector.tensor_tensor_scan(F2, of, F2, RESET,
                                         op0=Alu.add, op1=Alu.min)
            nc.vector.tensor_tensor_scan(F2[:, ::-1], ob[:, ::-1], F2[:, ::-1],
                                         RESET, op0=Alu.add, op1=Alu.min)

            # square in place (d1 <= 16, exact in fp16), then stage-2
            # transposes with a plain PSUM->SBUF copy
            nc.vector.tensor_tensor(Fst[:], Fst[:], Fst[:], op=Alu.mult)
            X0, X1 = RAD, RAD + W
            Dp = sb.tile([128, 2, SBK, W + 2 * RAD], dt.float16)
            nc.gpsimd.memset(Dp[:, :, :, 0:X0], PADV)
            nc.gpsimd.memset(Dp[:, :, :, X1:], PADV)
            for m in (0, 1):
                for sy in range(SBK):
                    tp = ps.tile([128, SBK, 128], dt.float16, tag="tp")
                    for sx in range(SBK):
                        nc.tensor.transpose(
                            tp[:, sx, :], Fst[:, m, sx, 128 * sy:128 * (sy + 1)],
                            ident[:])
                    nc.vector.tensor_copy(
                        Dp[:, m, sy, X0:X1],
                        tp[:].rearrange("p s x -> p (s x)"))

            # pass 2: windowed parabola min over |dx| <= 3 (exact)
            ms = {}
            for dx in range(1, RAD + 1):
                m_dx = sb.tile([128, 2, SBK, W], dt.float16, tag=f"m{dx}",
                               name=f"m{dx}")
                nc.vector.tensor_tensor(
                    m_dx[:], Dp[:, :, :, X0 - dx:X1 - dx],
                    Dp[:, :, :, X0 + dx:X1 + dx], op=Alu.min)
                ms[dx] = m_dx
            for dx in range(1, RAD + 1):
                nc.vector.tensor_scalar(ms[dx][:], ms[dx][:], float(dx * dx),
                                        None, op0=Alu.add)
            nc.vector.tensor_tensor(ms[1][:], ms[1][:], ms[2][:], op=Alu.min)
            nc.vector.tensor_tensor(ms[1][:], ms[1][:], ms[3][:], op=Alu.min)
            nc.vector.tensor_tensor(ms[1][:], ms[1][:], Dp[:, :, :, X0:X1],
                                    op=Alu.min)

        # sqrt(D)/3 via exp(0.5*ln(D) - ln3); u = exp(-(d0+d1)) in fp8
        acc = ms[1]
        nc.scalar.activation(acc[:], acc[:], Act.Ln, bias=eps_ap[:])
        nc.scalar.activation(acc[:], acc[:], Act.Exp, scale=0.5, bias=ln3_ap[:])
        dist = sb.tile([128, SBK, W], dt.float16)
        nc.vector.tensor_add(dist[:], acc[:, 0], acc[:, 1])
        u_t = sb.tile([128, SBK, W], dt.float16)
        nc.scalar.activation(u_t[:], dist[:], Act.Exp, scale=-1.0)
        u_dr = dr.tile([HW], dt.float16)
        with tc.tile_wait_until(ST_UW):
            nc.sync.dma_start(
                u_dr[:].rearrange("(sy y x) -> y sy x", sy=SBK, y=128, x=W),
                u_t[:])

        # =========================== CE ===========================
        acc_tot = None
        for s in range(SBK):
            pred_sb = pred_sbs[s]
            tbr = tbrs[s]
            expp = sb2.tile([120, JS], dt.bfloat16, tag="expp", bufs=2,
                            name=f"expp{s}")
            sg = ps.tile([Qs, 2, F], dt.float32, tag=f"sg{s}", bufs=1)
            for k in range(NEC):
                ck = slice(k * EC, (k + 1) * EC)
                nc.scalar.activation(expp[:, ck], pred_sb[:, ck], Act.Exp)
                nc.vector.tensor_scalar(tbr[:, ck], tbr[:, ck], iota120[:],
                                        None, op0=Alu.is_equal)
                eng = nc.gpsimd if (s, k) in POOL_MULT else nc.vector
                eng.tensor_tensor(tbr[:, ck], tbr[:, ck], expp[:, ck],
                                  op=Alu.mult)
                # matmuls for the F-chunks covered by this exp/oh chunk
                for i in range(k * (CHK // NEC), (k + 1) * (CHK // NEC)):
                    osl = ones_shift[:, XSH - G6 * i:XSH - G6 * i + Qs]
                    nc.tensor.matmul(sg[:, 0, :], osl,
                                     expp[:, i * F:(i + 1) * F],
                                     start=(i == 0), stop=(i == CHK - 1))
                    nc.tensor.matmul(sg[:, 1, :], osl,
                                     tbr[:, i * F:(i + 1) * F],
                                     start=(i == 0), stop=(i == CHK - 1))

            lsg = sb2.tile([Qs, 2, F], dt.float32, tag="lsg", bufs=2,
                           name=f"lsg{s}")
            nc.scalar.activation(lsg[:], sg[:], Act.Ln)
            ce_t = sb2.tile([Qs, F], dt.float32, tag="cet", bufs=2,
                            name=f"cet{s}")
            nc.vector.tensor_tensor(ce_t[:], lsg[:, 0, :], lsg[:, 1, :],
                                    op=Alu.subtract)

            u_sb = sb2.tile([Qs, F], dt.float16, tag="usb", bufs=2,
                            name=f"usb{s}")
            with tc.tile_wait_until(ST_UR[s]):
                nc.sync.dma_start(
                    u_sb[:],
                    u_dr[OFF[s]:OFF[s] + G6 * JS].rearrange(
                        "(g i f) -> i g f", g=G6, i=CHK, f=F))
            w_sb = sb2.tile([Qs, F], dt.float16, tag="wsb", bufs=2,
                            name=f"wsb{s}")
            nc.vector.tensor_scalar(w_sb[:], u_sb[:], 5.0, 1.0,
                                    op0=Alu.mult, op1=Alu.add)
            junk = sb2.tile([Qs, F], dt.float32, tag="junk", bufs=2,
                            name=f"junk{s}")
            acc_s = sb.tile([Qs, 1], dt.float32, tag=f"acc{s}",
                            name=f"acc{s}")
            nc.vector.scalar_tensor_tensor(
                junk[:], ce_t[:], 1.0, w_sb[:],
                op0=Alu.mult, op1=Alu.mult, accum_out=acc_s[:])
            if acc_tot is None:
                acc_tot = acc_s
            else:
                nc.vector.tensor_add(acc_tot[:], acc_tot[:], acc_s[:])

        nc.sync.dma_start(part_d.ap(), acc_tot[:])

    nc.compile()
    return nc


def kernel(pred, target):
    key = "nc"
    if key not in _CACHED:
        _CACHED[key] = build_nc()
    nc = _CACHED[key]
    consts = _consts()
    in_maps = []
    for b in range(N_CORES):
        in_maps.append({
            "pred": np.ascontiguousarray(pred[b], dtype=np.float32),
            "target": np.ascontiguousarray(target[b], dtype=np.int32),
            "ones_shift": consts["ones_shift"],
            "iota120": consts["iota120"],
            "ident": consts["ident"],
        })
    res = bass_utils.run_bass_kernel_spmd(
        nc, in_maps, core_ids=list(range(N_CORES)))
    total = 0.0
    for b in range(N_CORES):
        total += float(res.results[b]["partial"].astype(np.float64).sum())
    return np.float32(total / (N_CORES * HW))
